# revision 13
# baseline (speedup 1.0000x reference)
"""Distributed multi-head attention (RoPE) kernel for 8 TRN2 NeuronCores.

Problem: B=2, N=4096, C=512, H=8 heads, head_dim=64.
  qkv = x @ Wqkv.T + bqkv; q,k get interleaved RoPE;
  out = softmax(q k^T / sqrt(hd)) v; y = out @ Wproj.T + bproj.

Sharding: B*H = 16 (batch, head) units -> 2 heads per core (head-parallel,
batch split across core groups of 4). Each core computes a flash-style
attention for its 2 heads entirely on-chip (scores never hit DRAM) and a
partial output projection over its 128 channels; the host sums the 4 partial
projections per batch and adds bproj.

Per-core layout highlights:
  - q,k,v projections computed from xT (x transposed on host) so q/k land
    head-dim-major [d, n]; the 2 heads stacked on partitions 0-63 / 64-127.
  - RoPE pair-swap done with a 128x128 permutation matmul; rotation on DVE.
  - scores computed transposed S^T[key, query] (contraction dim 64, two heads
    run concurrently in separate row groups); exp on ACT (scale=1/8 fused);
    softmax denominator via an appended ones-column in the PV matmul lhsT.
  - normalization by 1/denom broadcast across partitions with a tiny K=2
    matmul, then the output projection accumulates both heads by row groups.
"""

import os
import warnings

warnings.filterwarnings("ignore")
os.environ.setdefault("JAX_PLATFORMS", "cpu")

import numpy as np

import concourse.bass as bass
import concourse.mybir as mybir
import concourse.tile as tile
from concourse import bacc
from concourse.alu_op_type import AluOpType
from concourse.bass_utils import run_bass_kernel_spmd

B, N, C = 2, 4096, 512
H = 8
HD = 64
SCALE = HD ** -0.5
THETA = 10000.0
NCORES = 8

F32 = mybir.dt.float32
BF16 = mybir.dt.bfloat16

NT = N // 512     # 8  column tiles of 512
NQT = N // 128    # 32 row tiles of 128
KT = N // 128     # 32 key tiles of 128
VW = 130          # per-key-tile v columns: [v_h1(64) | ones | v_h2(64)]

_CACHED_NC = None


def build_nc():
    nc = bacc.Bacc(None, target_bir_lowering=False, debug=False)

    xT_e = nc.declare_dram_parameter("xT", [C, N], F32, isOutput=False)
    wT_e = nc.declare_dram_parameter("wT", [C, 384], F32, isOutput=False)
    bq_e = nc.declare_dram_parameter("bq", [128, 1], F32, isOutput=False)
    bk_e = nc.declare_dram_parameter("bk", [128, 1], F32, isOutput=False)
    bv_e = nc.declare_dram_parameter("bv", [1, 128], F32, isOutput=False)
    wpT_e = nc.declare_dram_parameter("wpT", [64, 2 * C], F32, isOutput=False)
    cos_e = nc.declare_dram_parameter("cosf", [128, N], F32, isOutput=False)
    sin_e = nc.declare_dram_parameter("sinf", [128, N], F32, isOutput=False)
    psw_e = nc.declare_dram_parameter("pswap", [128, 128], F32, isOutput=False)
    out_e = nc.declare_dram_parameter("out", [N, C], F32, isOutput=True)

    Exp = mybir.ActivationFunctionType.Exp
    Recip = mybir.ActivationFunctionType.Reciprocal

    from contextlib import ExitStack

    with tile.TileContext(nc) as tc, ExitStack() as es:
        const = es.enter_context(tc.tile_pool(name="const", bufs=1))

        # ---- persistent SBUF tensors ----
        xb = [const.tile([128, N], BF16, name=f"xb{i}") for i in range(4)]
        wb = [const.tile([128, 384], BF16, name=f"wb{i}") for i in range(4)]
        cos_b = const.tile([128, N], BF16)
        sin_f = const.tile([128, N], F32)
        psw_b = const.tile([128, 128], BF16)
        bqf = const.tile([128, 1], F32)
        bkf = const.tile([128, 1], F32)
        bv_b = const.tile([1, 128], BF16)
        ones_b = const.tile([1, 128], BF16)
        qkraw = [const.tile([128, N], BF16, name=f"qkraw{i}") for i in range(2)]
        rope = [const.tile([128, N], BF16, name=f"rope{i}") for i in range(2)]
        v_sb = const.tile([128, KT * VW], BF16)
        outU = [const.tile([64, N], BF16, name=f"outU{i}") for i in range(2)]
        outN = [const.tile([64, N], BF16, name=f"outN{i}") for i in range(2)]
        denom = const.tile([1, 2 * N], F32)
        rcp_b = const.tile([1, 2 * N], BF16)
        wp_b = const.tile([64, 2 * C], BF16)

        # ---- load constants / inputs, cast to bf16 ----
        HN = N // 2
        with tc.tile_pool(name="stage", bufs=2) as stage:
            for ct in range(4):
                for hh in range(2):
                    hs = slice(hh * HN, (hh + 1) * HN)
                    xf = stage.tile([128, HN], F32, tag="xf")
                    nc.sync.dma_start(out=xf[:],
                                      in_=xT_e[ct * 128:(ct + 1) * 128, hs])
                    nc.vector.tensor_copy(xb[ct][:, hs], xf[:])
            for ct in range(4):
                wf = stage.tile([128, 384], F32, tag="wf")
                nc.sync.dma_start(out=wf[:], in_=wT_e[ct * 128:(ct + 1) * 128, :])
                nc.vector.tensor_copy(wb[ct][:], wf[:])
            for hh in range(2):
                hs = slice(hh * HN, (hh + 1) * HN)
                cf = stage.tile([128, HN], F32, tag="xf")
                nc.sync.dma_start(out=cf[:], in_=cos_e[:, hs])
                nc.vector.tensor_copy(cos_b[:, hs], cf[:])
            nc.sync.dma_start(out=sin_f[:], in_=sin_e[:])
            pf = stage.tile([128, 128], F32, tag="pf")
            nc.sync.dma_start(out=pf[:], in_=psw_e[:])
            nc.vector.tensor_copy(psw_b[:], pf[:])
            nc.sync.dma_start(out=bqf[:], in_=bq_e[:])
            nc.sync.dma_start(out=bkf[:], in_=bk_e[:])
            bvf = stage.tile([1, 128], F32, tag="ef")
            nc.sync.dma_start(out=bvf[:], in_=bv_e[:])
            nc.vector.tensor_copy(bv_b[:], bvf[:])
            wpf = stage.tile([64, 2 * C], F32, tag="xf")
            nc.sync.dma_start(out=wpf[:], in_=wpT_e[:])
            nc.vector.tensor_copy(wp_b[:], wpf[:])
            nc.vector.memset(ones_b[:], 1.0)
            nc.vector.memset(v_sb[:], 1.0)

        # ---- phase B: q,k projections (transposed layout) + phase C RoPE ----
        with tc.tile_pool(name="mm_ps", bufs=3, space="PSUM") as mm_ps, \
             tc.tile_pool(name="rtmp", bufs=4) as rtmp:
            for blk in range(2):  # 0=q, 1=k
                bias = bqf if blk == 0 else bkf
                for nt in range(NT):
                    ps = mm_ps.tile([128, 512], F32, tag="ps")
                    cs = slice(nt * 512, (nt + 1) * 512)
                    for ct in range(4):
                        nc.tensor.matmul(
                            ps[:],
                            wb[ct][:, blk * 128:(blk + 1) * 128],
                            xb[ct][:, cs],
                            start=(ct == 0),
                            stop=(ct == 3),
                        )
                    nc.vector.tensor_scalar_add(qkraw[blk][:, cs], ps[:], bias[:])
                # RoPE for this block
                for nt in range(NT):
                    cs = slice(nt * 512, (nt + 1) * 512)
                    ps_sw = mm_ps.tile([128, 512], F32, tag="ps")
                    nc.tensor.matmul(ps_sw[:], psw_b[:], qkraw[blk][:, cs],
                                     start=True, stop=True)
                    t1 = rtmp.tile([128, 512], BF16, tag="t1")
                    nc.vector.tensor_tensor(t1[:], qkraw[blk][:, cs],
                                            cos_b[:, cs], op=AluOpType.mult)
                    t2 = rtmp.tile([128, 512], BF16, tag="t2")
                    nc.vector.tensor_tensor(t2[:], ps_sw[:], sin_f[:, cs],
                                            op=AluOpType.mult)
                    nc.vector.tensor_tensor(rope[blk][:, cs], t1[:], t2[:],
                                            op=AluOpType.add)

            # ---- v projection (natural [key, channel] layout) ----
            for nt2 in range(KT):
                ps_v = mm_ps.tile([128, 512], F32, tag="ps")
                rs = slice(nt2 * 128, (nt2 + 1) * 128)
                nc.tensor.matmul(ps_v[:, 0:128], ones_b[:], bv_b[:],
                                 start=True, stop=False)
                for ct in range(4):
                    nc.tensor.matmul(
                        ps_v[:, 0:128],
                        xb[ct][:, rs],
                        wb[ct][:, 256:384],
                        start=False,
                        stop=(ct == 3),
                    )
                o = nt2 * VW
                nc.vector.tensor_copy(v_sb[:, o:o + 64], ps_v[:, 0:64])
                nc.vector.tensor_copy(v_sb[:, o + 65:o + 129], ps_v[:, 64:128])

        # ---- phase D: attention ----
        with tc.tile_pool(name="s_ps", bufs=2, space="PSUM") as s_ps, \
             tc.tile_pool(name="pv_ps", bufs=2, space="PSUM") as pv_ps, \
             tc.tile_pool(name="p_sb", bufs=3) as p_sb, \
             tc.tile_pool(name="dstage", bufs=2) as dstage:
            for qg in range(NT):
                qs = slice(qg * 512, (qg + 1) * 512)
                pv1 = pv_ps.tile([128, 512], F32, tag="pv1")
                pv2 = pv_ps.tile([128, 512], F32, tag="pv2")
                for kt in range(KT):
                    ks = slice(kt * 128, (kt + 1) * 128)
                    ps_s = s_ps.tile([128, 1024], F32, tag="s")
                    nc.tensor.matmul(ps_s[:, 0:512], rope[1][0:64, ks],
                                     rope[0][0:64, qs], start=True, stop=True,
                                     tile_position=(0, 0))
                    nc.tensor.matmul(ps_s[:, 512:1024], rope[1][64:128, ks],
                                     rope[0][64:128, qs], start=True, stop=True,
                                     tile_position=(64, 0))
                    p_t = p_sb.tile([128, 1024], BF16, tag="p")
                    nc.scalar.activation(p_t[:], ps_s[:], Exp, scale=SCALE)
                    o = kt * VW
                    nc.tensor.matmul(pv1[0:65, :], v_sb[:, o:o + 65],
                                     p_t[:, 0:512],
                                     start=(kt == 0), stop=(kt == KT - 1))
                    nc.tensor.matmul(pv2[0:65, :], v_sb[:, o + 65:o + 130],
                                     p_t[:, 512:1024],
                                     start=(kt == 0), stop=(kt == KT - 1))
                nc.vector.tensor_copy(outU[0][0:64, qs], pv1[0:64, :])
                nc.vector.tensor_copy(outU[1][0:64, qs], pv2[0:64, :])
                d1 = dstage.tile([65, 512], F32, tag="d1")
                nc.vector.tensor_copy(d1[64:65, :], pv1[64:65, :])
                nc.sync.dma_start(out=denom[0:1, qg * 512:(qg + 1) * 512],
                                  in_=d1[64:65, :])
                d2 = dstage.tile([65, 512], F32, tag="d2")
                nc.vector.tensor_copy(d2[64:65, :], pv2[64:65, :])
                nc.sync.dma_start(out=denom[0:1, N + qg * 512:N + (qg + 1) * 512],
                                  in_=d2[64:65, :])

        # ---- phase E: normalize + output projection ----
        with tc.tile_pool(name="e_ps", bufs=3, space="PSUM") as e_ps, \
             tc.tile_pool(name="o_sb", bufs=3) as o_sb:
            nc.scalar.activation(denom[:], denom[:],
                                 mybir.ActivationFunctionType.Ln)
            nc.scalar.activation(denom[:], denom[:], Exp, scale=-1.0)
            nc.vector.tensor_copy(rcp_b[:], denom[:])
            for qg in range(NT):
                qs = slice(qg * 512, (qg + 1) * 512)
                for h in range(2):
                    ps_bc = e_ps.tile([64, 512], F32, tag="bc")
                    nc.tensor.matmul(ps_bc[0:64, :], ones_b[0:1, 0:64],
                                     rcp_b[0:1, h * N + qg * 512:
                                           h * N + (qg + 1) * 512],
                                     start=True, stop=True)
                    nc.vector.tensor_tensor(outN[h][0:64, qs],
                                            outU[h][0:64, qs], ps_bc[0:64, :],
                                            op=AluOpType.mult)
            for qt in range(NQT):
                rs = slice(qt * 128, (qt + 1) * 128)
                ps_pr = e_ps.tile([128, 512], F32, tag="pr")
                nc.tensor.matmul(ps_pr[:], outN[0][0:64, rs], wp_b[0:64, 0:C],
                                 start=True, stop=False)
                nc.tensor.matmul(ps_pr[:], outN[1][0:64, rs], wp_b[0:64, C:2 * C],
                                 start=False, stop=True)
                o_t = o_sb.tile([128, C], F32, tag="o")
                nc.vector.tensor_copy(o_t[:], ps_pr[:])
                nc.sync.dma_start(out=out_e[rs, :], in_=o_t[:])

    nc.finalize()
    return nc


def _rope_tables():
    freqs = 1.0 / THETA ** (np.arange(0, HD, 2, dtype=np.float64) / HD)  # [32]
    t = np.arange(N, dtype=np.float64)
    ang = np.outer(t, freqs)  # [N, 32]
    cos64 = np.repeat(np.cos(ang).T, 2, axis=0)  # [64, N]
    sin_ = np.sin(ang).T
    sin64 = np.empty((HD, N), dtype=np.float64)
    sin64[0::2] = -sin_
    sin64[1::2] = sin_
    cosf = np.tile(cos64, (2, 1)).astype(np.float32)  # [128, N]
    sinf = np.tile(sin64, (2, 1)).astype(np.float32)
    return np.ascontiguousarray(cosf), np.ascontiguousarray(sinf)


def make_in_maps(x, Wqkv, bqkv, Wproj):
    cosf, sinf = _rope_tables()
    pswap = np.zeros((128, 128), dtype=np.float32)
    idx = np.arange(0, 128, 2)
    pswap[idx, idx + 1] = 1.0
    pswap[idx + 1, idx] = 1.0

    xTs = [np.ascontiguousarray(x[b].T).astype(np.float32) for b in range(B)]
    in_maps = []
    for c in range(NCORES):
        b, hp = c // 4, c % 4
        r0 = hp * 128
        W_c = np.concatenate(
            [Wqkv[r0:r0 + 128], Wqkv[512 + r0:512 + r0 + 128],
             Wqkv[1024 + r0:1024 + r0 + 128]], axis=0)  # [384, 512]
        wT = np.ascontiguousarray(W_c.T).astype(np.float32)
        bq = bqkv[r0:r0 + 128].reshape(128, 1).astype(np.float32)
        bk = bqkv[512 + r0:512 + r0 + 128].reshape(128, 1).astype(np.float32)
        bv = bqkv[1024 + r0:1024 + r0 + 128].reshape(1, 128).astype(np.float32)
        wp1 = Wproj[:, r0:r0 + 64].T          # [64, 512] head h1 channels
        wp2 = Wproj[:, r0 + 64:r0 + 128].T    # [64, 512] head h2 channels
        wpT = np.ascontiguousarray(
            np.concatenate([wp1, wp2], axis=1)).astype(np.float32)
        in_maps.append({
            "xT": xTs[b], "wT": wT, "bq": bq, "bk": bk, "bv": bv,
            "wpT": wpT, "cosf": cosf, "sinf": sinf, "pswap": pswap,
        })
    return in_maps


def kernel(x, Wqkv, bqkv, Wproj, bproj, _trace=False):
    global _CACHED_NC
    if _CACHED_NC is None:
        _CACHED_NC = build_nc()
    nc = _CACHED_NC
    x = np.asarray(x, dtype=np.float32)
    Wqkv = np.asarray(Wqkv, dtype=np.float32)
    bqkv = np.asarray(bqkv, dtype=np.float32)
    Wproj = np.asarray(Wproj, dtype=np.float32)
    bproj = np.asarray(bproj, dtype=np.float32)

    in_maps = make_in_maps(x, Wqkv, bqkv, Wproj)
    res = run_bass_kernel_spmd(nc, in_maps, core_ids=list(range(NCORES)),
                               trace=_trace)
    out = np.empty((B, N, C), dtype=np.float32)
    for b in range(B):
        acc = res.results[4 * b]["out"].astype(np.float32).copy()
        for i in range(1, 4):
            acc += res.results[4 * b + i]["out"]
        out[b] = acc + bproj[None, :]
    if _trace:
        return out, res
    return out


# revision 14
# speedup vs baseline: 1.0547x; 1.0547x over previous
"""Distributed multi-head attention (RoPE) kernel for 8 TRN2 NeuronCores.

Problem: B=2, N=4096, C=512, H=8 heads, head_dim=64.
  qkv = x @ Wqkv.T + bqkv; q,k get interleaved RoPE;
  out = softmax(q k^T / sqrt(hd)) v; y = out @ Wproj.T + bproj.

Sharding: B*H = 16 (batch, head) units -> 2 heads per core (head-parallel,
batch split across core groups of 4). Each core computes a flash-style
attention for its 2 heads entirely on-chip (scores never hit DRAM) and a
partial output projection over its 128 channels; the host sums the 4 partial
projections per batch and adds bproj.

Per-core layout highlights:
  - q,k,v projections computed from xT (x transposed on host) so q/k land
    head-dim-major [d, n]; the 2 heads stacked on partitions 0-63 / 64-127.
  - RoPE pair-swap done with a 128x128 permutation matmul; rotation on DVE.
  - scores computed transposed S^T[key, query] (contraction dim 64, two heads
    run concurrently in separate row groups); exp on ACT (scale=1/8 fused);
    softmax denominator via an appended ones-column in the PV matmul lhsT.
  - normalization by 1/denom broadcast across partitions with a tiny K=2
    matmul, then the output projection accumulates both heads by row groups.
"""

import os
import warnings

warnings.filterwarnings("ignore")
os.environ.setdefault("JAX_PLATFORMS", "cpu")

import numpy as np

import concourse.bass as bass
import concourse.mybir as mybir
import concourse.tile as tile
from concourse import bacc
from concourse.alu_op_type import AluOpType
from concourse.bass_utils import run_bass_kernel_spmd

B, N, C = 2, 4096, 512
H = 8
HD = 64
SCALE = HD ** -0.5
THETA = 10000.0
NCORES = 8

F32 = mybir.dt.float32
BF16 = mybir.dt.bfloat16

NT = N // 512     # 8  column tiles of 512
NQT = N // 128    # 32 row tiles of 128
KT = N // 128     # 32 key tiles of 128
VW = 130          # per-key-tile v columns: [v_h1(64) | ones | v_h2(64)]

_CACHED_NC = None


def build_nc():
    nc = bacc.Bacc(None, target_bir_lowering=False, debug=False)

    xT_e = nc.declare_dram_parameter("xT", [C, N], BF16, isOutput=False)
    wT_e = nc.declare_dram_parameter("wT", [C, 384], BF16, isOutput=False)
    bq_e = nc.declare_dram_parameter("bq", [128, 1], F32, isOutput=False)
    bk_e = nc.declare_dram_parameter("bk", [128, 1], F32, isOutput=False)
    bv_e = nc.declare_dram_parameter("bv", [1, 128], F32, isOutput=False)
    wpT_e = nc.declare_dram_parameter("wpT", [64, 2 * C], BF16, isOutput=False)
    cos_e = nc.declare_dram_parameter("cosf", [128, N], BF16, isOutput=False)
    sin_e = nc.declare_dram_parameter("sinf", [128, N], F32, isOutput=False)
    psw_e = nc.declare_dram_parameter("pswap", [128, 128], BF16, isOutput=False)
    out_e = nc.declare_dram_parameter("out", [N, C], F32, isOutput=True)

    Exp = mybir.ActivationFunctionType.Exp
    Recip = mybir.ActivationFunctionType.Reciprocal

    from contextlib import ExitStack

    with tile.TileContext(nc) as tc, ExitStack() as es:
        const = es.enter_context(tc.tile_pool(name="const", bufs=1))

        # ---- persistent SBUF tensors ----
        xb = [const.tile([128, N], BF16, name=f"xb{i}") for i in range(4)]
        wb = [const.tile([128, 384], BF16, name=f"wb{i}") for i in range(4)]
        cos_b = const.tile([128, N], BF16)
        sin_f = const.tile([128, N], F32)
        psw_b = const.tile([128, 128], BF16)
        bqf = const.tile([128, 1], F32)
        bkf = const.tile([128, 1], F32)
        bv_b = const.tile([1, 128], BF16)
        ones_b = const.tile([1, 128], BF16)
        qkraw = [const.tile([128, N], BF16, name=f"qkraw{i}") for i in range(2)]
        rope = [const.tile([128, N], BF16, name=f"rope{i}") for i in range(2)]
        v_sb = const.tile([128, KT * VW], BF16)
        outU = [const.tile([64, N], BF16, name=f"outU{i}") for i in range(2)]
        outN = [const.tile([64, N], BF16, name=f"outN{i}") for i in range(2)]
        denom = const.tile([1, 2 * N], F32)
        rcp_b = const.tile([1, 2 * N], BF16)
        wp_b = const.tile([64, 2 * C], BF16)

        # ---- load constants / inputs (weights arrive pre-cast to bf16) ----
        with tc.tile_pool(name="stage", bufs=2) as stage:
            for ct in range(4):
                nc.sync.dma_start(out=xb[ct][:],
                                  in_=xT_e[ct * 128:(ct + 1) * 128, :])
                nc.sync.dma_start(out=wb[ct][:],
                                  in_=wT_e[ct * 128:(ct + 1) * 128, :])
            nc.sync.dma_start(out=cos_b[:], in_=cos_e[:])
            nc.sync.dma_start(out=sin_f[:], in_=sin_e[:])
            nc.sync.dma_start(out=psw_b[:], in_=psw_e[:])
            nc.sync.dma_start(out=bqf[:], in_=bq_e[:])
            nc.sync.dma_start(out=bkf[:], in_=bk_e[:])
            bvf = stage.tile([1, 128], F32, tag="ef")
            nc.sync.dma_start(out=bvf[:], in_=bv_e[:])
            nc.vector.tensor_copy(bv_b[:], bvf[:])
            nc.sync.dma_start(out=wp_b[:], in_=wpT_e[:])
            nc.vector.memset(ones_b[:], 1.0)
            nc.vector.memset(v_sb[:], 1.0)

        # ---- phase B: q,k projections (transposed layout) + phase C RoPE ----
        with tc.tile_pool(name="mm_ps", bufs=3, space="PSUM") as mm_ps, \
             tc.tile_pool(name="rtmp", bufs=4) as rtmp:
            for blk in range(2):  # 0=q, 1=k
                bias = bqf if blk == 0 else bkf
                for nt in range(NT):
                    ps = mm_ps.tile([128, 512], F32, tag="ps")
                    cs = slice(nt * 512, (nt + 1) * 512)
                    for ct in range(4):
                        nc.tensor.matmul(
                            ps[:],
                            wb[ct][:, blk * 128:(blk + 1) * 128],
                            xb[ct][:, cs],
                            start=(ct == 0),
                            stop=(ct == 3),
                        )
                    nc.vector.tensor_scalar_add(qkraw[blk][:, cs], ps[:], bias[:])
                # RoPE for this block
                for nt in range(NT):
                    cs = slice(nt * 512, (nt + 1) * 512)
                    ps_sw = mm_ps.tile([128, 512], F32, tag="ps")
                    nc.tensor.matmul(ps_sw[:], psw_b[:], qkraw[blk][:, cs],
                                     start=True, stop=True)
                    t1 = rtmp.tile([128, 512], BF16, tag="t1")
                    nc.vector.tensor_tensor(t1[:], qkraw[blk][:, cs],
                                            cos_b[:, cs], op=AluOpType.mult)
                    t2 = rtmp.tile([128, 512], BF16, tag="t2")
                    nc.vector.tensor_tensor(t2[:], ps_sw[:], sin_f[:, cs],
                                            op=AluOpType.mult)
                    nc.vector.tensor_tensor(rope[blk][:, cs], t1[:], t2[:],
                                            op=AluOpType.add)

            # ---- v projection (natural [key, channel] layout) ----
            for nt2 in range(KT):
                ps_v = mm_ps.tile([128, 512], F32, tag="ps")
                rs = slice(nt2 * 128, (nt2 + 1) * 128)
                nc.tensor.matmul(ps_v[:, 0:128], ones_b[:], bv_b[:],
                                 start=True, stop=False)
                for ct in range(4):
                    nc.tensor.matmul(
                        ps_v[:, 0:128],
                        xb[ct][:, rs],
                        wb[ct][:, 256:384],
                        start=False,
                        stop=(ct == 3),
                    )
                o = nt2 * VW
                nc.vector.tensor_copy(v_sb[:, o:o + 64], ps_v[:, 0:64])
                nc.vector.tensor_copy(v_sb[:, o + 65:o + 129], ps_v[:, 64:128])

        # ---- phase D: attention ----
        with tc.tile_pool(name="s_ps", bufs=2, space="PSUM") as s_ps, \
             tc.tile_pool(name="pv_ps", bufs=2, space="PSUM") as pv_ps, \
             tc.tile_pool(name="p_sb", bufs=3) as p_sb, \
             tc.tile_pool(name="dstage", bufs=2) as dstage:
            for qg in range(NT):
                qs = slice(qg * 512, (qg + 1) * 512)
                pv1 = pv_ps.tile([128, 512], F32, tag="pv1")
                pv2 = pv_ps.tile([128, 512], F32, tag="pv2")
                for kt in range(KT):
                    ks = slice(kt * 128, (kt + 1) * 128)
                    ps_s = s_ps.tile([128, 1024], F32, tag="s")
                    nc.tensor.matmul(ps_s[:, 0:512], rope[1][0:64, ks],
                                     rope[0][0:64, qs], start=True, stop=True,
                                     tile_position=(0, 0))
                    nc.tensor.matmul(ps_s[:, 512:1024], rope[1][64:128, ks],
                                     rope[0][64:128, qs], start=True, stop=True,
                                     tile_position=(64, 0))
                    p_t = p_sb.tile([128, 1024], BF16, tag="p")
                    nc.scalar.activation(p_t[:], ps_s[:], Exp, scale=SCALE)
                    o = kt * VW
                    nc.tensor.matmul(pv1[0:65, :], v_sb[:, o:o + 65],
                                     p_t[:, 0:512],
                                     start=(kt == 0), stop=(kt == KT - 1))
                    nc.tensor.matmul(pv2[0:65, :], v_sb[:, o + 65:o + 130],
                                     p_t[:, 512:1024],
                                     start=(kt == 0), stop=(kt == KT - 1))
                nc.vector.tensor_copy(outU[0][0:64, qs], pv1[0:64, :])
                nc.vector.tensor_copy(outU[1][0:64, qs], pv2[0:64, :])
                d1 = dstage.tile([65, 512], F32, tag="d1")
                nc.vector.tensor_copy(d1[64:65, :], pv1[64:65, :])
                nc.sync.dma_start(out=denom[0:1, qg * 512:(qg + 1) * 512],
                                  in_=d1[64:65, :])
                d2 = dstage.tile([65, 512], F32, tag="d2")
                nc.vector.tensor_copy(d2[64:65, :], pv2[64:65, :])
                nc.sync.dma_start(out=denom[0:1, N + qg * 512:N + (qg + 1) * 512],
                                  in_=d2[64:65, :])

        # ---- phase E: normalize + output projection ----
        with tc.tile_pool(name="e_ps", bufs=3, space="PSUM") as e_ps, \
             tc.tile_pool(name="o_sb", bufs=3) as o_sb:
            nc.scalar.activation(denom[:], denom[:],
                                 mybir.ActivationFunctionType.Ln)
            nc.scalar.activation(denom[:], denom[:], Exp, scale=-1.0)
            nc.vector.tensor_copy(rcp_b[:], denom[:])
            for qg in range(NT):
                qs = slice(qg * 512, (qg + 1) * 512)
                for h in range(2):
                    ps_bc = e_ps.tile([64, 512], F32, tag="bc")
                    nc.tensor.matmul(ps_bc[0:64, :], ones_b[0:1, 0:64],
                                     rcp_b[0:1, h * N + qg * 512:
                                           h * N + (qg + 1) * 512],
                                     start=True, stop=True)
                    nc.vector.tensor_tensor(outN[h][0:64, qs],
                                            outU[h][0:64, qs], ps_bc[0:64, :],
                                            op=AluOpType.mult)
            for qt in range(NQT):
                rs = slice(qt * 128, (qt + 1) * 128)
                ps_pr = e_ps.tile([128, 512], F32, tag="pr")
                nc.tensor.matmul(ps_pr[:], outN[0][0:64, rs], wp_b[0:64, 0:C],
                                 start=True, stop=False)
                nc.tensor.matmul(ps_pr[:], outN[1][0:64, rs], wp_b[0:64, C:2 * C],
                                 start=False, stop=True)
                o_t = o_sb.tile([128, C], F32, tag="o")
                nc.vector.tensor_copy(o_t[:], ps_pr[:])
                nc.sync.dma_start(out=out_e[rs, :], in_=o_t[:])

    nc.finalize()
    return nc


def _rope_tables():
    freqs = 1.0 / THETA ** (np.arange(0, HD, 2, dtype=np.float64) / HD)  # [32]
    t = np.arange(N, dtype=np.float64)
    ang = np.outer(t, freqs)  # [N, 32]
    cos64 = np.repeat(np.cos(ang).T, 2, axis=0)  # [64, N]
    sin_ = np.sin(ang).T
    sin64 = np.empty((HD, N), dtype=np.float64)
    sin64[0::2] = -sin_
    sin64[1::2] = sin_
    cosf = np.tile(cos64, (2, 1)).astype(np.float32)  # [128, N]
    sinf = np.tile(sin64, (2, 1)).astype(np.float32)
    return np.ascontiguousarray(cosf), np.ascontiguousarray(sinf)


def make_in_maps(x, Wqkv, bqkv, Wproj):
    cosf, sinf = _rope_tables()
    pswap = np.zeros((128, 128), dtype=np.float32)
    idx = np.arange(0, 128, 2)
    pswap[idx, idx + 1] = 1.0
    pswap[idx + 1, idx] = 1.0

    import ml_dtypes
    bf16 = ml_dtypes.bfloat16
    xTs = [np.ascontiguousarray(x[b].T).astype(bf16) for b in range(B)]
    in_maps = []
    for c in range(NCORES):
        b, hp = c // 4, c % 4
        r0 = hp * 128
        W_c = np.concatenate(
            [Wqkv[r0:r0 + 128], Wqkv[512 + r0:512 + r0 + 128],
             Wqkv[1024 + r0:1024 + r0 + 128]], axis=0)  # [384, 512]
        wT = np.ascontiguousarray(W_c.T).astype(bf16)
        bq = bqkv[r0:r0 + 128].reshape(128, 1).astype(np.float32)
        bk = bqkv[512 + r0:512 + r0 + 128].reshape(128, 1).astype(np.float32)
        bv = bqkv[1024 + r0:1024 + r0 + 128].reshape(1, 128).astype(np.float32)
        wp1 = Wproj[:, r0:r0 + 64].T          # [64, 512] head h1 channels
        wp2 = Wproj[:, r0 + 64:r0 + 128].T    # [64, 512] head h2 channels
        wpT = np.ascontiguousarray(
            np.concatenate([wp1, wp2], axis=1)).astype(bf16)
        in_maps.append({
            "xT": xTs[b], "wT": wT, "bq": bq, "bk": bk, "bv": bv,
            "wpT": wpT, "cosf": cosf.astype(bf16), "sinf": sinf,
            "pswap": pswap.astype(bf16),
        })
    return in_maps


def kernel(x, Wqkv, bqkv, Wproj, bproj, _trace=False):
    global _CACHED_NC
    if _CACHED_NC is None:
        _CACHED_NC = build_nc()
    nc = _CACHED_NC
    x = np.asarray(x, dtype=np.float32)
    Wqkv = np.asarray(Wqkv, dtype=np.float32)
    bqkv = np.asarray(bqkv, dtype=np.float32)
    Wproj = np.asarray(Wproj, dtype=np.float32)
    bproj = np.asarray(bproj, dtype=np.float32)

    in_maps = make_in_maps(x, Wqkv, bqkv, Wproj)
    res = run_bass_kernel_spmd(nc, in_maps, core_ids=list(range(NCORES)),
                               trace=_trace)
    out = np.empty((B, N, C), dtype=np.float32)
    for b in range(B):
        acc = res.results[4 * b]["out"].astype(np.float32).copy()
        for i in range(1, 4):
            acc += res.results[4 * b + i]["out"]
        out[b] = acc + bproj[None, :]
    if _trace:
        return out, res
    return out
